# revision 1
# baseline (speedup 1.0000x reference)
"""RNN-T JointNetwork Trainium2 kernel.

logits[b,t,u,v] = sum_j W_out[v,j] * tanh(f[b,t,j] + g[b,u,j]) + b_out[v]
  f = enc_out @ W_enc.T   [B,T,640]
  g = pred_out @ W_pred.T [B,U,640]

Sharding: data-parallel over B=8 across the 8 NeuronCores (1 batch/core).

The full f32 logits are 536 MB; the axon tunnel fetches at ~50 MB/s, so
the wall clock is dominated by output transfer, not device time. The
kernel therefore returns int8-quantized logits (per-(t,u)-row absmax
scales, 16.8 MB/core) plus the f32 scales; the host dequantizes into the
final f32 array. Row-absmax int8 adds ~0.9% L2 error (gate is 2e-2).

Per-core device program (everything resident on-chip):
  phase 1: fT = W_enc @ enc.T -> [640,256] f32 accumulated in PSUM (stays
           there; ScalarE reads PSUM faster than SBUF), gT -> [640,64]
           copied to SBUF (activation bias operands must be SBUF).
           Inputs bf16 (host-cast) so phase 1 runs at full PE rate.
  phase 2: per u: combT_u[j,t] = tanh(fT + gT[:,u]) via ScalarE activation
           with per-partition bias (u-major ordering turns the broadcast
           into a partition-axis bias), output cast to bf16
  phase 3: logits rows = combT_u.T @ W_outT in bf16, K=640 as 5x128 chunks
           accumulated into a [128,1024] PSUM tile (two 512-col bank
           groups)
  phase 4: VectorE adds bias (bf16 out), absmax-reduces each row,
           reciprocal -> 127/m; ScalarE scales rows to int8
           (round-to-nearest, saturating); DMA int8 rows out t-major
           ([T,U,V], 1 KB contiguous per row) so the host dequant
           multiply is contiguous. Scales accumulate in SBUF and leave
           in a single DMA at the end.
"""

import os
import sys

for _p in ("/opt/trn_rl_repo",):
    if _p not in sys.path:
        sys.path.insert(0, _p)

import numpy as np
import ml_dtypes


def _enable_jax_compile_cache():
    """Persistent XLA executable cache: skips the per-process
    HLO->walrus->NEFF compile of the wrapped bass_exec call (~1s) when
    warm. jax is pre-imported by the site hook, so set config directly;
    cache errors are non-fatal to jax."""
    try:
        cc = os.path.expanduser("~/.cache/jax_bass_cc")
        os.makedirs(cc, exist_ok=True)
        import jax

        jax.config.update("jax_compilation_cache_dir", cc)
        jax.config.update("jax_persistent_cache_min_compile_time_secs", 0.0)
    except Exception:
        pass

B, T, U = 8, 256, 64
D_ENC, D_PRED, D_JOINT, VOCAB = 512, 512, 640, 1024
KE = D_ENC // 128   # 4 contraction chunks for enc/pred matmuls
KJ = D_JOINT // 128  # 5 contraction chunks for the vocab matmul
N_CORES = 8
RT = T // 128  # 2 row tiles per u

_compiled = None


def _build():
    import concourse.bacc as bacc
    import concourse.bass as bass
    import concourse.mybir as mybir
    import concourse.tile as tile

    f32 = mybir.dt.float32
    bf16 = mybir.dt.bfloat16
    i8 = mybir.dt.int8
    PSUM = bass.MemorySpace.PSUM
    tanh = mybir.ActivationFunctionType.Tanh
    copy_f = mybir.ActivationFunctionType.Copy

    nc = bacc.Bacc(
        "TRN2",
        target_bir_lowering=False,
        debug=False,
        enable_asserts=False,
    )

    enc_d = nc.dram_tensor("enc", [128, KE, T], bf16, kind="ExternalInput")
    pred_d = nc.dram_tensor("pred", [128, KE, U], bf16, kind="ExternalInput")
    wenc_d = nc.dram_tensor("wenc", [128, KE, D_JOINT], bf16, kind="ExternalInput")
    wpred_d = nc.dram_tensor("wpred", [128, KE, D_JOINT], bf16, kind="ExternalInput")
    wout_d = nc.dram_tensor("wout", [128, KJ, VOCAB], bf16, kind="ExternalInput")
    bias_d = nc.dram_tensor("bias", [1, VOCAB], f32, kind="ExternalInput")
    out_d = nc.dram_tensor("out", [T, U, VOCAB], i8, kind="ExternalOutput")
    sc_d = nc.dram_tensor("sc", [128, RT, U], f32, kind="ExternalOutput")

    with tile.TileContext(nc) as tc:
        with (
            tc.tile_pool(name="const", bufs=1) as const,
            tc.tile_pool(name="comb", bufs=3) as comb_pool,
            tc.tile_pool(name="outsb", bufs=4) as out_pool,
            tc.tile_pool(name="qsb", bufs=4) as q_pool,
            tc.tile_pool(name="msb", bufs=4) as m_pool,
            tc.tile_pool(name="psf", bufs=1, space=PSUM) as psf,
        ):
            # Trigger the Tanh ACT table load before any data arrives.
            warm = const.tile([1, 8], f32)
            warm2 = const.tile([1, 8], f32)
            nc.vector.memset(warm[:], 0.0)
            nc.scalar.activation(warm2[:], warm[:], tanh)

            pred_sb = const.tile([128, KE, U], bf16)
            wpred_sb = const.tile([128, KE, D_JOINT], bf16)
            enc_sb = const.tile([128, KE, T], bf16)
            wenc_sb = const.tile([128, KE, D_JOINT], bf16)
            wout_sb = const.tile([128, KJ, VOCAB], bf16)
            bias_row = const.tile([1, VOCAB], f32)
            bias_sb = const.tile([128, VOCAB], f32)
            ones_sb = const.tile([1, 128], f32)
            gT_sb = const.tile([128, KJ, U], f32)
            sc_sb = const.tile([128, RT, U], f32)
            fT_ps = psf.tile([128, KJ, T], f32)  # 5 KiB/partition -> 3 banks

            # PE warmup: dummy matmuls on zeroed data while input DMAs are
            # in flight, so HAM un-throttles before the real matmuls start.
            wz = const.tile([128, 512], bf16)
            nc.vector.memset(wz[:], 0.0)
            nc.vector.memset(ones_sb[:], 1.0)

            # Input DMA triggers spread across the three DMA-capable
            # engines so they issue in parallel.
            nc.sync.dma_start(pred_sb[:], pred_d[:])
            nc.gpsimd.dma_start(wpred_sb[:], wpred_d[:])
            nc.scalar.dma_start(enc_sb[:], enc_d[:])
            nc.sync.dma_start(wenc_sb[:], wenc_d[:])
            nc.gpsimd.dma_start(wout_sb[:], wout_d[:])
            nc.scalar.dma_start(bias_row[:], bias_d[:])

            with tc.tile_pool(name="psw", bufs=1, space=PSUM) as psw:
                pw = psw.tile([128, 512], f32)
                for i in range(10):
                    nc.tensor.matmul(pw[:], wz[:, :128], wz[:], start=True, stop=True)

            # phase 1: j-outer accumulation groups (a group must fully
            # close before another start=True touches its PSUM bank);
            # gT copies interleave under the following fT matmul group.
            with tc.tile_pool(name="psg", bufs=2, space=PSUM) as psg:
                for j in range(KJ):
                    ps = psg.tile([128, U], f32, tag="psg")
                    for k in range(KE):
                        nc.tensor.matmul(
                            ps[:],
                            wpred_sb[:, k, j * 128:(j + 1) * 128],
                            pred_sb[:, k, :],
                            start=(k == 0),
                            stop=(k == KE - 1),
                        )
                    nc.scalar.copy(gT_sb[:, j, :], ps[:])
                    for k in range(KE):
                        nc.tensor.matmul(
                            fT_ps[:, j, :],
                            wenc_sb[:, k, j * 128:(j + 1) * 128],
                            enc_sb[:, k, :],
                            start=(k == 0),
                            stop=(k == KE - 1),
                        )

                # replicate b_out across partitions with two rank-1 matmuls
                bps = psg.tile([128, 512], f32, tag="psg", name="bps")
                nc.tensor.matmul(bps[:], ones_sb[:], bias_row[:, 0:512],
                                 start=True, stop=True)
                nc.vector.tensor_copy(bias_sb[:, 0:512], bps[:])
                bps2 = psg.tile([128, 512], f32, tag="psg", name="bps2")
                nc.tensor.matmul(bps2[:], ones_sb[:], bias_row[:, 512:1024],
                                 start=True, stop=True)
                nc.vector.tensor_copy(bias_sb[:, 512:1024], bps2[:])

            with tc.tile_pool(name="pso", bufs=2, space=PSUM) as pso:
                for u in range(U):
                    comb = comb_pool.tile([128, KJ, T], bf16, tag="comb")
                    for j in range(KJ):
                        nc.scalar.activation(
                            comb[:, j, :],
                            fT_ps[:, j, :],
                            tanh,
                            bias=gT_sb[:, j, u:u + 1],
                        )
                    for rt in range(RT):
                        # [128,1024] f32 = 2 PSUM banks; each 512-col half
                        # is its own accumulation group in its own bank.
                        po = pso.tile([128, VOCAB], f32, tag="pso")
                        ob = out_pool.tile([128, VOCAB], bf16, tag="ob")
                        q = q_pool.tile([128, VOCAB], i8, tag="q")
                        mt = m_pool.tile([128, 2], f32, tag="mt")
                        rows = slice(rt * 128, (rt + 1) * 128)
                        for j in range(KJ):
                            lhsT = comb[:, j, rows]
                            nc.tensor.matmul(
                                po[:, 0:512], lhsT, wout_sb[:, j, 0:512],
                                start=(j == 0), stop=(j == KJ - 1),
                            )
                            nc.tensor.matmul(
                                po[:, 512:1024], lhsT, wout_sb[:, j, 512:1024],
                                start=(j == 0), stop=(j == KJ - 1),
                            )
                        # bias add f32 PSUM -> bf16 SBUF (bf16 is what gets
                        # quantized so scale and payload stay consistent)
                        nc.vector.tensor_add(ob[:], po[:], bias_sb[:])
                        # row absmax -> scales SBUF (shipped once at end)
                        nc.vector.tensor_reduce(
                            sc_sb[:, rt, u:u + 1], ob[:],
                            axis=mybir.AxisListType.X,
                            op=mybir.AluOpType.max,
                            apply_absolute_value=True,
                        )
                        # mt0 = max(m/127, tiny)  (guard against zero rows)
                        nc.vector.tensor_scalar(
                            mt[:, 0:1], sc_sb[:, rt, u:u + 1],
                            1.0 / 127.0, 1e-30,
                            op0=mybir.AluOpType.mult,
                            op1=mybir.AluOpType.max,
                        )
                        nc.vector.reciprocal(mt[:, 1:2], mt[:, 0:1])
                        # int8 rows: round-to-nearest saturating downconvert
                        nc.scalar.activation(
                            q[:], ob[:], copy_f, scale=mt[:, 1:2],
                        )
                        nc.sync.dma_start(out_d[rows, u, :], q[:])

            nc.gpsimd.dma_start(sc_d[:], sc_sb[:])

    nc.compile()
    return nc


def _get_compiled():
    global _compiled
    if _compiled is None:
        _compiled = _build()
    return _compiled


def _prep_inputs(enc_out, pred_out, W_enc, W_pred, W_out, b_out):
    bf = ml_dtypes.bfloat16
    enc_out = np.asarray(enc_out, dtype=np.float32)
    pred_out = np.asarray(pred_out, dtype=np.float32)
    W_enc = np.asarray(W_enc, dtype=np.float32)
    W_pred = np.asarray(W_pred, dtype=np.float32)
    W_out = np.asarray(W_out, dtype=np.float32)
    b_out = np.asarray(b_out, dtype=np.float32)

    # [d, x] -> [128, d//128, x]: partition-major chunking of the d axis
    wenc = np.ascontiguousarray(
        W_enc.T.reshape(KE, 128, D_JOINT).transpose(1, 0, 2)).astype(bf)
    wpred = np.ascontiguousarray(
        W_pred.T.reshape(KE, 128, D_JOINT).transpose(1, 0, 2)).astype(bf)
    wout = np.ascontiguousarray(
        W_out.T.reshape(KJ, 128, VOCAB).transpose(1, 0, 2)).astype(bf)
    bias = np.ascontiguousarray(b_out.reshape(1, VOCAB))

    in_maps = []
    for b in range(B):
        encb = np.ascontiguousarray(
            enc_out[b].T.reshape(KE, 128, T).transpose(1, 0, 2)).astype(bf)
        predb = np.ascontiguousarray(
            pred_out[b].T.reshape(KE, 128, U).transpose(1, 0, 2)).astype(bf)
        in_maps.append({
            "enc": encb, "pred": predb, "wenc": wenc, "wpred": wpred,
            "wout": wout, "bias": bias,
        })
    return in_maps


def run(inputs, trace=False, **kwargs):
    from concourse.bass_utils import run_bass_kernel_spmd

    _enable_jax_compile_cache()
    nc = _get_compiled()
    in_maps = _prep_inputs(**inputs)
    res = run_bass_kernel_spmd(
        nc, in_maps, core_ids=list(range(N_CORES)), trace=trace, **kwargs)
    out = np.empty((B, T, U, VOCAB), np.float32)
    for b in range(B):
        q = res.results[b]["out"]                   # [T, U, V] int8
        m = res.results[b]["sc"]                    # [128, RT, U] f32
        s = m.transpose(1, 0, 2).reshape(T, U) * np.float32(1.0 / 127.0)
        np.multiply(q, s[:, :, None], out=out[b])
    return out, res


def kernel(**inputs):
    out, _ = run(inputs, trace=False)
    return out



# revision 4
# speedup vs baseline: 1.0989x; 1.0989x over previous
"""RNN-T JointNetwork Trainium2 kernel.

logits[b,t,u,v] = sum_j W_out[v,j] * tanh(f[b,t,j] + g[b,u,j]) + b_out[v]
  f = enc_out @ W_enc.T   [B,T,640]
  g = pred_out @ W_pred.T [B,U,640]

Sharding: data-parallel over B=8 across the 8 NeuronCores (1 batch/core).

v3 design notes (vs the int8 baseline at 342 us):
  The baseline saturated all three compute engines (~90% each): the
  per-row int8 absmax/scale machinery cost VectorE+ScalarE as much time
  as the matmul itself, and tanh read fT from PSUM in f32 (ScalarE reads
  f32 at half rate). This version outputs fp16 (skips absmax entirely)
  and feeds tanh from fp16 SBUF, leaving the PE as the only near-
  saturated engine (bf16/fp16 matmul roofline ~277 us).

  phase 1: fT[j,t] (=f.T) accumulated in PSUM then drained to SBUF fp16;
           gT[j,u] drained f32 (used as tanh bias operand).
  phase 2: comb[j,uu,t] = tanh(fT + gT[:,u]) via ScalarE activation with
           per-partition bias, fp16 out; one tile per u-PAIR so a matmul
           can stream 512 moving columns (2 u x 256 t).
  phase 3: vocab-on-partition orientation: for each (ublock of 8 u, vt):
           out_ps[v,(uu,t)] += wout[j,vt-chunk].T @ comb[j] with j outer,
           pair inner -> each weight load feeds 4 N=512 matmuls (vs 1:2
           in the t-orientation), 4 psum banks x 2 generations = 8 banks.
  phase 4: VectorE drains each bank with ONE fused op: fp16 out =
           psum_f32 + bias_v (tensor_scalar add, per-partition bias),
           then DMA [128v, 2u x 256t] -> HBM [vt,v,u,t], 1 KiB contiguous
           per partition. Host transposes to [T,U,V] and upcasts.

  All matmul operands are fp16 (PE upconverts to e10m11 internally, so
  fp16 is strictly more accurate than bf16 at identical speed).
"""

import os
import sys

for _p in ("/opt/trn_rl_repo",):
    if _p not in sys.path:
        sys.path.insert(0, _p)

import numpy as np


def _enable_jax_compile_cache():
    """Persistent XLA executable cache: skips the per-process
    HLO->walrus->NEFF compile of the wrapped bass_exec call (~1s) when
    warm. jax is pre-imported by the site hook, so set config directly;
    cache errors are non-fatal to jax."""
    try:
        cc = os.path.expanduser("~/.cache/jax_bass_cc")
        os.makedirs(cc, exist_ok=True)
        import jax

        jax.config.update("jax_compilation_cache_dir", cc)
        jax.config.update("jax_persistent_cache_min_compile_time_secs", 0.0)
    except Exception:
        pass

B, T, U = 8, 256, 64
D_ENC, D_PRED, D_JOINT, VOCAB = 512, 512, 640, 1024
KE = D_ENC // 128   # 4 contraction chunks for enc/pred matmuls
KJ = D_JOINT // 128  # 5 contraction chunks for the vocab matmul
N_CORES = 8
VT = VOCAB // 128   # 8 vocab partition tiles
UB = 8              # u's per ublock
NP = UB // 2        # u-pairs per ublock

_compiled = None


def _build():
    import concourse.bacc as bacc
    import concourse.bass as bass
    import concourse.mybir as mybir
    import concourse.tile as tile

    f32 = mybir.dt.float32
    f16 = mybir.dt.float16
    PSUM = bass.MemorySpace.PSUM
    tanh = mybir.ActivationFunctionType.Tanh
    add = mybir.AluOpType.add

    nc = bacc.Bacc(
        "TRN2",
        target_bir_lowering=False,
        debug=False,
        enable_asserts=False,
    )

    enc_d = nc.dram_tensor("enc", [128, KE, T], f16, kind="ExternalInput")
    pred_d = nc.dram_tensor("pred", [128, KE, U], f16, kind="ExternalInput")
    wenc_d = nc.dram_tensor("wenc", [128, KE, D_JOINT], f16, kind="ExternalInput")
    wpred_d = nc.dram_tensor("wpred", [128, KE, D_JOINT], f16, kind="ExternalInput")
    wout_d = nc.dram_tensor("wout", [128, KJ, VOCAB], f16, kind="ExternalInput")
    bias_d = nc.dram_tensor("bias", [128, VT], f32, kind="ExternalInput")
    out_d = nc.dram_tensor("out", [VT, 128, U, T], f16, kind="ExternalOutput")

    out_qs = None  # round-robin DMA trigger engines for output tiles

    with tile.TileContext(nc) as tc:
        with (
            tc.tile_pool(name="const", bufs=1) as const,
            tc.tile_pool(name="comb", bufs=2) as comb_pool,
            tc.tile_pool(name="stage", bufs=3) as stage_pool,
        ):
            # Trigger the Tanh ACT table load before any data arrives.
            warm = const.tile([1, 8], f32)
            warm2 = const.tile([1, 8], f32)
            nc.vector.memset(warm[:], 0.0)
            nc.scalar.activation(warm2[:], warm[:], tanh)

            enc_sb = const.tile([128, KE, T], f16)
            pred_sb = const.tile([128, KE, U], f16)
            wenc_sb = const.tile([128, KE, D_JOINT], f16)
            wpred_sb = const.tile([128, KE, D_JOINT], f16)
            wout_sb = const.tile([128, KJ, VOCAB], f16)
            bias_sb = const.tile([128, VT], f32)
            fT_sb = const.tile([128, KJ, T], f16)
            gT_sb = const.tile([128, KJ, U], f32)

            # PE warmup: dummy matmuls on zeroed data while input DMAs are
            # in flight, so HAM un-throttles before the real matmuls start.
            wz = const.tile([128, 512], f16)
            nc.vector.memset(wz[:], 0.0)

            # Input DMA triggers spread across the three DMA-capable
            # engines; enc/wenc first (phase-1 critical path).
            nc.sync.dma_start(enc_sb[:], enc_d[:])
            nc.gpsimd.dma_start(wenc_sb[:], wenc_d[:])
            nc.scalar.dma_start(pred_sb[:], pred_d[:])
            nc.scalar.dma_start(wpred_sb[:], wpred_d[:])
            nc.sync.dma_start(wout_sb[:], wout_d[:])
            nc.gpsimd.dma_start(bias_sb[:], bias_d[:])

            with tc.tile_pool(name="psw", bufs=1, space=PSUM) as psw:
                pw = psw.tile([128, 512], f32)
                for i in range(10):
                    nc.tensor.matmul(pw[:], wz[:, :128], wz[:], start=True, stop=True)

            # phase 1: per j-chunk, fT (needed first by phase 2) then gT.
            with tc.tile_pool(name="psg", bufs=2, space=PSUM) as psg:
                for j in range(KJ):
                    psf = psg.tile([128, T], f32, tag="psf")
                    for k in range(KE):
                        nc.tensor.matmul(
                            psf[:],
                            wenc_sb[:, k, j * 128:(j + 1) * 128],
                            enc_sb[:, k, :],
                            start=(k == 0),
                            stop=(k == KE - 1),
                        )
                    nc.vector.tensor_copy(fT_sb[:, j, :], psf[:])
                    psp = psg.tile([128, U], f32, tag="psp")
                    for k in range(KE):
                        nc.tensor.matmul(
                            psp[:],
                            wpred_sb[:, k, j * 128:(j + 1) * 128],
                            pred_sb[:, k, :],
                            start=(k == 0),
                            stop=(k == KE - 1),
                        )
                    nc.scalar.copy(gT_sb[:, j, :], psp[:])

            def make_comb(ub):
                """Emit the 40 tanh activations for one ublock of 8 u's;
                returns the 4 u-pair tiles [128, KJ, 2, T] fp16."""
                tiles = []
                for p in range(NP):
                    cp = comb_pool.tile([128, KJ, 2, T], f16, tag=f"comb{p}")
                    for uu in range(2):
                        u = ub * UB + p * 2 + uu
                        for j in range(KJ):
                            nc.scalar.activation(
                                cp[:, j, uu, :],
                                fT_sb[:, j, :],
                                tanh,
                                bias=gT_sb[:, j, u:u + 1],
                            )
                    tiles.append(cp)
                return tiles

            with tc.tile_pool(name="pso", bufs=2, space=PSUM) as pso:
                combs = make_comb(0)
                for ub in range(8):
                    combs_next = make_comb(ub + 1) if ub < 7 else None
                    for vt in range(VT):
                        pss = [
                            pso.tile([128, 512], f32, tag=f"ps{p}",
                                     name=f"ps{p}_{ub}_{vt}")
                            for p in range(NP)
                        ]
                        for j in range(KJ):
                            w = wout_sb[:, j, vt * 128:(vt + 1) * 128]
                            for p in range(NP):
                                nc.tensor.matmul(
                                    pss[p][:],
                                    w,
                                    combs[p][:, j, :, :],
                                    start=(j == 0),
                                    stop=(j == KJ - 1),
                                )
                        for p in range(NP):
                            st = stage_pool.tile([128, 512], f16, tag=f"st{p}")
                            nc.vector.tensor_scalar(
                                st[:], pss[p][:], bias_sb[:, vt:vt + 1], None,
                                op0=add,
                            )
                            u0 = ub * UB + p * 2
                            eng = (nc.sync, nc.gpsimd)[(vt + p) % 2]
                            eng.dma_start(out_d[vt, :, u0:u0 + 2, :], st[:])
                    combs = combs_next

    nc.compile()
    return nc


def _get_compiled():
    global _compiled
    if _compiled is None:
        _compiled = _build()
    return _compiled


def _prep_inputs(enc_out, pred_out, W_enc, W_pred, W_out, b_out):
    f16 = np.float16
    enc_out = np.asarray(enc_out, dtype=np.float32)
    pred_out = np.asarray(pred_out, dtype=np.float32)
    W_enc = np.asarray(W_enc, dtype=np.float32)
    W_pred = np.asarray(W_pred, dtype=np.float32)
    W_out = np.asarray(W_out, dtype=np.float32)
    b_out = np.asarray(b_out, dtype=np.float32)

    # [d, x] -> [128, d//128, x]: partition-major chunking of the d axis
    wenc = np.ascontiguousarray(
        W_enc.T.reshape(KE, 128, D_JOINT).transpose(1, 0, 2)).astype(f16)
    wpred = np.ascontiguousarray(
        W_pred.T.reshape(KE, 128, D_JOINT).transpose(1, 0, 2)).astype(f16)
    wout = np.ascontiguousarray(
        W_out.T.reshape(KJ, 128, VOCAB).transpose(1, 0, 2)).astype(f16)
    bias = np.ascontiguousarray(b_out.reshape(VT, 128).T).astype(np.float32)

    in_maps = []
    for b in range(B):
        encb = np.ascontiguousarray(
            enc_out[b].T.reshape(KE, 128, T).transpose(1, 0, 2)).astype(f16)
        predb = np.ascontiguousarray(
            pred_out[b].T.reshape(KE, 128, U).transpose(1, 0, 2)).astype(f16)
        in_maps.append({
            "enc": encb, "pred": predb, "wenc": wenc, "wpred": wpred,
            "wout": wout, "bias": bias,
        })
    return in_maps


def run(inputs, trace=False, **kwargs):
    from concourse.bass_utils import run_bass_kernel_spmd

    _enable_jax_compile_cache()
    nc = _get_compiled()
    in_maps = _prep_inputs(**inputs)
    res = run_bass_kernel_spmd(
        nc, in_maps, core_ids=list(range(N_CORES)), trace=trace, **kwargs)
    out = np.empty((B, T, U, VOCAB), np.float32)
    for b in range(B):
        q = res.results[b]["out"]                   # [VT, 128, U, T] fp16
        # out[b][t, u, vt*128+v] = q[vt, v, u, t]
        out[b] = q.transpose(3, 2, 0, 1).reshape(T, U, VOCAB)
    return out, res


def kernel(**inputs):
    out, _ = run(inputs, trace=False)
    return out


# revision 9
# speedup vs baseline: 1.1104x; 1.0105x over previous
"""RNN-T JointNetwork Trainium2 kernel.

logits[b,t,u,v] = sum_j W_out[v,j] * tanh(f[b,t,j] + g[b,u,j]) + b_out[v]
  f = enc_out @ W_enc.T   [B,T,640]
  g = pred_out @ W_pred.T [B,U,640]

Sharding: data-parallel over B=8 across the 8 NeuronCores (1 batch/core).

v3 design notes (vs the int8 baseline at 342 us):
  The baseline saturated all three compute engines (~90% each): the
  per-row int8 absmax/scale machinery cost VectorE+ScalarE as much time
  as the matmul itself, and tanh read fT from PSUM in f32 (ScalarE reads
  f32 at half rate). This version outputs fp16 (skips absmax entirely)
  and feeds tanh from fp16 SBUF, leaving the PE as the only near-
  saturated engine (bf16/fp16 matmul roofline ~277 us).

  phase 1: fT[j,t] (=f.T) accumulated in PSUM then drained to SBUF fp16;
           gT[j,u] drained f32 (used as tanh bias operand).
  phase 2: comb[j,uu,t] = tanh(fT + gT[:,u]) via ScalarE activation with
           per-partition bias, fp16 out; one tile per u-PAIR so a matmul
           can stream 512 moving columns (2 u x 256 t).
  phase 3: vocab-on-partition orientation: for each (ublock of 8 u, vt):
           out_ps[v,(uu,t)] += wout[j,vt-chunk].T @ comb[j] with j outer,
           pair inner -> each weight load feeds 4 N=512 matmuls (vs 1:2
           in the t-orientation), 4 psum banks x 2 generations = 8 banks.
  phase 4: VectorE drains each bank with ONE fused op: fp16 out =
           psum_f32 + bias_v (tensor_scalar add, per-partition bias),
           then DMA [128v, 2u x 256t] -> HBM [vt,v,u,t], 1 KiB contiguous
           per partition. Host transposes to [T,U,V] and upcasts.

  All matmul operands are fp16 (PE upconverts to e10m11 internally, so
  fp16 is strictly more accurate than bf16 at identical speed).
"""

import os
import sys

for _p in ("/opt/trn_rl_repo",):
    if _p not in sys.path:
        sys.path.insert(0, _p)

import numpy as np


def _enable_jax_compile_cache():
    """Persistent XLA executable cache: skips the per-process
    HLO->walrus->NEFF compile of the wrapped bass_exec call (~1s) when
    warm. jax is pre-imported by the site hook, so set config directly;
    cache errors are non-fatal to jax."""
    try:
        cc = os.path.expanduser("~/.cache/jax_bass_cc")
        os.makedirs(cc, exist_ok=True)
        import jax

        jax.config.update("jax_compilation_cache_dir", cc)
        jax.config.update("jax_persistent_cache_min_compile_time_secs", 0.0)
    except Exception:
        pass

B, T, U = 8, 256, 64
D_ENC, D_PRED, D_JOINT, VOCAB = 512, 512, 640, 1024
KE = D_ENC // 128   # 4 contraction chunks for enc/pred matmuls
KJ = D_JOINT // 128  # 5 contraction chunks for the vocab matmul
N_CORES = 8
VT = VOCAB // 128   # 8 vocab partition tiles
UB = 8              # u's per ublock
NP = UB // 2        # u-pairs per ublock

_compiled = None


def _build():
    import concourse.bacc as bacc
    import concourse.bass as bass
    import concourse.mybir as mybir
    import concourse.tile as tile

    f32 = mybir.dt.float32
    f16 = mybir.dt.float16
    PSUM = bass.MemorySpace.PSUM
    tanh = mybir.ActivationFunctionType.Tanh
    add = mybir.AluOpType.add

    nc = bacc.Bacc(
        "TRN2",
        target_bir_lowering=False,
        debug=False,
        enable_asserts=False,
    )

    enc_d = nc.dram_tensor("enc", [128, KE, T], f16, kind="ExternalInput")
    pred_d = nc.dram_tensor("pred", [128, KE, U], f16, kind="ExternalInput")
    wenc_d = nc.dram_tensor("wenc", [128, KJ, KE, 128], f16, kind="ExternalInput")
    wpred_d = nc.dram_tensor("wpred", [128, KJ, KE, 128], f16, kind="ExternalInput")
    wout_d = nc.dram_tensor("wout", [128, VT, KJ, 128], f16, kind="ExternalInput")
    bias_d = nc.dram_tensor("bias", [128, VT], f32, kind="ExternalInput")
    out_d = nc.dram_tensor("out", [VT, 128, U, T], f16, kind="ExternalOutput")

    out_qs = None  # round-robin DMA trigger engines for output tiles

    with tile.TileContext(nc) as tc:
        with (
            tc.tile_pool(name="const", bufs=1) as const,
            tc.tile_pool(name="comb", bufs=2) as comb_pool,
            tc.tile_pool(name="stage", bufs=3) as stage_pool,
        ):
            # Trigger the Tanh ACT table load before any data arrives.
            warm = const.tile([1, 8], f32)
            warm2 = const.tile([1, 8], f32)
            nc.vector.memset(warm[:], 0.0)
            nc.scalar.activation(warm2[:], warm[:], tanh)

            enc_sb = const.tile([128, KE, T], f16)
            pred_sb = const.tile([128, KE, U], f16)
            wenc_sb = const.tile([128, KJ, KE, 128], f16)
            wpred_sb = const.tile([128, KJ, KE, 128], f16)
            wout_sb = const.tile([128, VT, KJ, 128], f16)
            bias_sb = const.tile([128, VT], f32)
            fT_sb = const.tile([128, KJ, T], f16)
            gT_sb = const.tile([128, KJ, U], f32)

            # PE warmup: dummy matmuls on zeroed data while input DMAs are
            # in flight, so HAM un-throttles before the real matmuls start.
            wz = const.tile([128, 512], f16)
            nc.vector.memset(wz[:], 0.0)

            # Input DMA triggers spread across the three DMA-capable
            # engines, chunked so each consumer can start on its first
            # chunk: phase-1 j-group needs enc + wenc[:, j]; phase-3
            # vt-group needs wout[:, vt].
            nc.sync.dma_start(enc_sb[:], enc_d[:])
            nc.scalar.dma_start(pred_sb[:], pred_d[:])
            for j in range(KJ):
                nc.gpsimd.dma_start(wenc_sb[:, j], wenc_d[:, j])
                nc.scalar.dma_start(wpred_sb[:, j], wpred_d[:, j])
            for vt in range(VT):
                nc.sync.dma_start(wout_sb[:, vt], wout_d[:, vt])
            nc.gpsimd.dma_start(bias_sb[:], bias_d[:])

            with tc.tile_pool(name="psw", bufs=1, space=PSUM) as psw:
                pw = psw.tile([128, 512], f32)
                for i in range(10):
                    nc.tensor.matmul(pw[:], wz[:, :128], wz[:], start=True, stop=True)

            # phase 1: per j-chunk, fT (needed first by phase 2) then gT.
            with tc.tile_pool(name="psg", bufs=2, space=PSUM) as psg:
                for j in range(KJ):
                    psf = psg.tile([128, T], f32, tag="psf")
                    for k in range(KE):
                        nc.tensor.matmul(
                            psf[:],
                            wenc_sb[:, j, k, :],
                            enc_sb[:, k, :],
                            start=(k == 0),
                            stop=(k == KE - 1),
                        )
                    nc.vector.tensor_copy(fT_sb[:, j, :], psf[:])
                    psp = psg.tile([128, U], f32, tag="psp")
                    for k in range(KE):
                        nc.tensor.matmul(
                            psp[:],
                            wpred_sb[:, j, k, :],
                            pred_sb[:, k, :],
                            start=(k == 0),
                            stop=(k == KE - 1),
                        )
                    nc.scalar.copy(gT_sb[:, j, :], psp[:])

            def make_comb(ub):
                """Emit the 40 tanh activations for one ublock of 8 u's;
                returns the 4 u-pair tiles [128, KJ, 2, T] fp16."""
                tiles = []
                for p in range(NP):
                    cp = comb_pool.tile([128, KJ, 2, T], f16, tag=f"comb{p}")
                    for uu in range(2):
                        u = ub * UB + p * 2 + uu
                        for j in range(KJ):
                            nc.scalar.activation(
                                cp[:, j, uu, :],
                                fT_sb[:, j, :],
                                tanh,
                                bias=gT_sb[:, j, u:u + 1],
                            )
                    tiles.append(cp)
                return tiles

            with tc.tile_pool(name="pso", bufs=2, space=PSUM) as pso:
                combs = make_comb(0)
                for ub in range(8):
                    combs_next = make_comb(ub + 1) if ub < 7 else None
                    for vt in range(VT):
                        pss = [
                            pso.tile([128, 512], f32, tag=f"ps{p}",
                                     name=f"ps{p}_{ub}_{vt}")
                            for p in range(NP)
                        ]
                        for j in range(KJ):
                            w = wout_sb[:, vt, j, :]
                            for p in range(NP):
                                nc.tensor.matmul(
                                    pss[p][:],
                                    w,
                                    combs[p][:, j, :, :],
                                    start=(j == 0),
                                    stop=(j == KJ - 1),
                                )
                        for p in range(NP):
                            st = stage_pool.tile([128, 512], f16, tag=f"st{p}")
                            nc.vector.tensor_scalar(
                                st[:], pss[p][:], bias_sb[:, vt:vt + 1], None,
                                op0=add,
                            )
                            u0 = ub * UB + p * 2
                            eng = (nc.sync, nc.gpsimd)[(vt + p) % 2]
                            eng.dma_start(out_d[vt, :, u0:u0 + 2, :], st[:])
                    combs = combs_next

    nc.compile()
    return nc


def _get_compiled():
    global _compiled
    if _compiled is None:
        _compiled = _build()
    return _compiled


def _prep_inputs(enc_out, pred_out, W_enc, W_pred, W_out, b_out):
    f16 = np.float16
    enc_out = np.asarray(enc_out, dtype=np.float32)
    pred_out = np.asarray(pred_out, dtype=np.float32)
    W_enc = np.asarray(W_enc, dtype=np.float32)
    W_pred = np.asarray(W_pred, dtype=np.float32)
    W_out = np.asarray(W_out, dtype=np.float32)
    b_out = np.asarray(b_out, dtype=np.float32)

    # W_enc.T is [d, j]; chunk both axes by 128 -> [128 dp, KJ, KE, 128 jq]
    # (wenc[p, j, k, q] = W_enc[j*128+q, k*128+p]) so the per-j slice a
    # phase-1 group consumes is one contiguous DMA.
    wenc = np.ascontiguousarray(
        W_enc.T.reshape(KE, 128, KJ, 128).transpose(1, 2, 0, 3)).astype(f16)
    wpred = np.ascontiguousarray(
        W_pred.T.reshape(KE, 128, KJ, 128).transpose(1, 2, 0, 3)).astype(f16)
    # W_out.T is [j, v]; -> [128 jp, VT, KJ, 128 vq] so per-vt slices are
    # contiguous DMAs.
    wout = np.ascontiguousarray(
        W_out.T.reshape(KJ, 128, VT, 128).transpose(1, 2, 0, 3)).astype(f16)
    bias = np.ascontiguousarray(b_out.reshape(VT, 128).T).astype(np.float32)

    in_maps = []
    for b in range(B):
        encb = np.ascontiguousarray(
            enc_out[b].T.reshape(KE, 128, T).transpose(1, 0, 2)).astype(f16)
        predb = np.ascontiguousarray(
            pred_out[b].T.reshape(KE, 128, U).transpose(1, 0, 2)).astype(f16)
        in_maps.append({
            "enc": encb, "pred": predb, "wenc": wenc, "wpred": wpred,
            "wout": wout, "bias": bias,
        })
    return in_maps


def run(inputs, trace=False, **kwargs):
    from concourse.bass_utils import run_bass_kernel_spmd

    _enable_jax_compile_cache()
    nc = _get_compiled()
    in_maps = _prep_inputs(**inputs)
    res = run_bass_kernel_spmd(
        nc, in_maps, core_ids=list(range(N_CORES)), trace=trace, **kwargs)
    out = np.empty((B, T, U, VOCAB), np.float32)
    for b in range(B):
        q = res.results[b]["out"]                   # [VT, 128, U, T] fp16
        # out[b][t, u, vt*128+v] = q[vt, v, u, t]
        out[b] = q.transpose(3, 2, 0, 1).reshape(T, U, VOCAB)
    return out, res


def kernel(**inputs):
    out, _ = run(inputs, trace=False)
    return out


# revision 11
# speedup vs baseline: 1.1114x; 1.0009x over previous
"""RNN-T JointNetwork Trainium2 kernel.

logits[b,t,u,v] = sum_j W_out[v,j] * tanh(f[b,t,j] + g[b,u,j]) + b_out[v]
  f = enc_out @ W_enc.T   [B,T,640]
  g = pred_out @ W_pred.T [B,U,640]

Sharding: data-parallel over B=8 across the 8 NeuronCores (1 batch/core).

v3 design notes (vs the int8 baseline at 342 us):
  The baseline saturated all three compute engines (~90% each): the
  per-row int8 absmax/scale machinery cost VectorE+ScalarE as much time
  as the matmul itself, and tanh read fT from PSUM in f32 (ScalarE reads
  f32 at half rate). This version outputs fp16 (skips absmax entirely)
  and feeds tanh from fp16 SBUF, leaving the PE as the only near-
  saturated engine (bf16/fp16 matmul roofline ~277 us).

  phase 1: fT[j,t] (=f.T) accumulated in PSUM then drained to SBUF fp16;
           gT[j,u] drained f32 (used as tanh bias operand).
  phase 2: comb[j,uu,t] = tanh(fT + gT[:,u]) via ScalarE activation with
           per-partition bias, fp16 out; one tile per u-PAIR so a matmul
           can stream 512 moving columns (2 u x 256 t).
  phase 3: vocab-on-partition orientation: for each (ublock of 8 u, vt):
           out_ps[v,(uu,t)] += wout[j,vt-chunk].T @ comb[j] with j outer,
           pair inner -> each weight load feeds 4 N=512 matmuls (vs 1:2
           in the t-orientation), 4 psum banks x 2 generations = 8 banks.
  phase 4: VectorE drains each bank with ONE fused op: fp16 out =
           psum_f32 + bias_v (tensor_scalar add, per-partition bias),
           then DMA [128v, 2u x 256t] -> HBM [vt,v,u,t], 1 KiB contiguous
           per partition. Host transposes to [T,U,V] and upcasts.

  All matmul operands are fp16 (PE upconverts to e10m11 internally, so
  fp16 is strictly more accurate than bf16 at identical speed).
"""

import os
import sys

for _p in ("/opt/trn_rl_repo",):
    if _p not in sys.path:
        sys.path.insert(0, _p)

import numpy as np


def _enable_jax_compile_cache():
    """Persistent XLA executable cache: skips the per-process
    HLO->walrus->NEFF compile of the wrapped bass_exec call (~1s) when
    warm. jax is pre-imported by the site hook, so set config directly;
    cache errors are non-fatal to jax."""
    try:
        cc = os.path.expanduser("~/.cache/jax_bass_cc")
        os.makedirs(cc, exist_ok=True)
        import jax

        jax.config.update("jax_compilation_cache_dir", cc)
        jax.config.update("jax_persistent_cache_min_compile_time_secs", 0.0)
    except Exception:
        pass

B, T, U = 8, 256, 64
D_ENC, D_PRED, D_JOINT, VOCAB = 512, 512, 640, 1024
KE = D_ENC // 128   # 4 contraction chunks for enc/pred matmuls
KJ = D_JOINT // 128  # 5 contraction chunks for the vocab matmul
N_CORES = 8
VT = VOCAB // 128   # 8 vocab partition tiles
UB = 8              # u's per ublock
NP = UB // 2        # u-pairs per ublock

_compiled = None


def _build():
    import concourse.bacc as bacc
    import concourse.bass as bass
    import concourse.mybir as mybir
    import concourse.tile as tile

    f32 = mybir.dt.float32
    f16 = mybir.dt.float16
    PSUM = bass.MemorySpace.PSUM
    tanh = mybir.ActivationFunctionType.Tanh
    add = mybir.AluOpType.add

    nc = bacc.Bacc(
        "TRN2",
        target_bir_lowering=False,
        debug=False,
        enable_asserts=False,
    )

    enc_d = nc.dram_tensor("enc", [128, KE, T], f16, kind="ExternalInput")
    pred_d = nc.dram_tensor("pred", [128, KE, U], f16, kind="ExternalInput")
    wenc_d = nc.dram_tensor("wenc", [128, KJ, KE, 128], f16, kind="ExternalInput")
    wpred_d = nc.dram_tensor("wpred", [128, KJ, KE, 128], f16, kind="ExternalInput")
    wout_d = nc.dram_tensor("wout", [128, VT, KJ, 128], f16, kind="ExternalInput")
    bias_d = nc.dram_tensor("bias", [128, VT], f32, kind="ExternalInput")
    out_d = nc.dram_tensor("out", [VT, 128, U, T], f16, kind="ExternalOutput")

    out_qs = None  # round-robin DMA trigger engines for output tiles

    with tile.TileContext(nc) as tc:
        with (
            tc.tile_pool(name="const", bufs=1) as const,
            tc.tile_pool(name="comb", bufs=2) as comb_pool,
            tc.tile_pool(name="stage", bufs=3) as stage_pool,
        ):
            # Trigger the Tanh ACT table load before any data arrives.
            warm = const.tile([1, 8], f32)
            warm2 = const.tile([1, 8], f32)
            nc.vector.memset(warm[:], 0.0)
            nc.scalar.activation(warm2[:], warm[:], tanh)

            enc_sb = const.tile([128, KE, T], f16)
            pred_sb = const.tile([128, KE, U], f16)
            wenc_sb = const.tile([128, KJ, KE, 128], f16)
            wpred_sb = const.tile([128, KJ, KE, 128], f16)
            wout_sb = const.tile([128, VT, KJ, 128], f16)
            bias_sb = const.tile([128, VT], f32)
            fT_sb = const.tile([128, KJ, T], f16)
            gT_sb = const.tile([128, KJ, U], f32)

            # PE warmup: dummy matmuls on zeroed data while input DMAs are
            # in flight, so HAM un-throttles before the real matmuls start.
            wz = const.tile([128, 512], f16)
            nc.gpsimd.memset(wz[:], 0.0)

            # Input DMA triggers on sync+gpsimd only (scalar's FIFO must
            # stay clear for the tanh stream), chunked so each phase-1
            # j-group starts on its chunk's arrival. wout is one big DMA
            # on scalar (single trigger, needed only at phase-3 start).
            nc.sync.dma_start(enc_sb[:], enc_d[:])
            nc.gpsimd.dma_start(pred_sb[:], pred_d[:])
            nc.scalar.dma_start(wout_sb[:], wout_d[:])
            for j in range(KJ):
                nc.sync.dma_start(wenc_sb[:, j], wenc_d[:, j])
                nc.gpsimd.dma_start(wpred_sb[:, j], wpred_d[:, j])
            nc.gpsimd.dma_start(bias_sb[:], bias_d[:])

            with tc.tile_pool(name="psw", bufs=1, space=PSUM) as psw:
                pw = psw.tile([128, 512], f32)
                for i in range(4):
                    nc.tensor.matmul(pw[:], wz[:, :128], wz[:], start=True, stop=True)

            # phase 1: per j-chunk, fT (needed first by phase 2) then gT.
            with tc.tile_pool(name="psg", bufs=2, space=PSUM) as psg:
                for j in range(KJ):
                    psf = psg.tile([128, T], f32, tag="psf")
                    for k in range(KE):
                        nc.tensor.matmul(
                            psf[:],
                            wenc_sb[:, j, k, :],
                            enc_sb[:, k, :],
                            start=(k == 0),
                            stop=(k == KE - 1),
                        )
                    nc.vector.tensor_copy(fT_sb[:, j, :], psf[:])
                    psp = psg.tile([128, U], f32, tag="psp")
                    for k in range(KE):
                        nc.tensor.matmul(
                            psp[:],
                            wpred_sb[:, j, k, :],
                            pred_sb[:, k, :],
                            start=(k == 0),
                            stop=(k == KE - 1),
                        )
                    nc.vector.tensor_copy(gT_sb[:, j, :], psp[:])

            def make_comb(ub):
                """Emit the 40 tanh activations for one ublock of 8 u's;
                returns the 4 u-pair tiles [128, KJ, 2, T] fp16."""
                tiles = []
                for p in range(NP):
                    cp = comb_pool.tile([128, KJ, 2, T], f16, tag=f"comb{p}")
                    for uu in range(2):
                        u = ub * UB + p * 2 + uu
                        for j in range(KJ):
                            nc.scalar.activation(
                                cp[:, j, uu, :],
                                fT_sb[:, j, :],
                                tanh,
                                bias=gT_sb[:, j, u:u + 1],
                            )
                    tiles.append(cp)
                return tiles

            with tc.tile_pool(name="pso", bufs=2, space=PSUM) as pso:
                combs = make_comb(0)
                for ub in range(8):
                    combs_next = make_comb(ub + 1) if ub < 7 else None
                    for vt in range(VT):
                        pss = [
                            pso.tile([128, 512], f32, tag=f"ps{p}",
                                     name=f"ps{p}_{ub}_{vt}")
                            for p in range(NP)
                        ]
                        for j in range(KJ):
                            w = wout_sb[:, vt, j, :]
                            for p in range(NP):
                                nc.tensor.matmul(
                                    pss[p][:],
                                    w,
                                    combs[p][:, j, :, :],
                                    start=(j == 0),
                                    stop=(j == KJ - 1),
                                )
                        for p in range(NP):
                            st = stage_pool.tile([128, 512], f16, tag=f"st{p}")
                            nc.vector.tensor_scalar(
                                st[:], pss[p][:], bias_sb[:, vt:vt + 1], None,
                                op0=add,
                            )
                            u0 = ub * UB + p * 2
                            eng = (nc.sync, nc.gpsimd)[(vt + p) % 2]
                            eng.dma_start(out_d[vt, :, u0:u0 + 2, :], st[:])
                    combs = combs_next

    nc.compile()
    return nc


def _get_compiled():
    global _compiled
    if _compiled is None:
        _compiled = _build()
    return _compiled


def _prep_inputs(enc_out, pred_out, W_enc, W_pred, W_out, b_out):
    f16 = np.float16
    enc_out = np.asarray(enc_out, dtype=np.float32)
    pred_out = np.asarray(pred_out, dtype=np.float32)
    W_enc = np.asarray(W_enc, dtype=np.float32)
    W_pred = np.asarray(W_pred, dtype=np.float32)
    W_out = np.asarray(W_out, dtype=np.float32)
    b_out = np.asarray(b_out, dtype=np.float32)

    # W_enc.T is [d, j]; chunk both axes by 128 -> [128 dp, KJ, KE, 128 jq]
    # (wenc[p, j, k, q] = W_enc[j*128+q, k*128+p]) so the per-j slice a
    # phase-1 group consumes is one contiguous DMA.
    wenc = np.ascontiguousarray(
        W_enc.T.reshape(KE, 128, KJ, 128).transpose(1, 2, 0, 3)).astype(f16)
    wpred = np.ascontiguousarray(
        W_pred.T.reshape(KE, 128, KJ, 128).transpose(1, 2, 0, 3)).astype(f16)
    # W_out.T is [j, v]; -> [128 jp, VT, KJ, 128 vq] so per-vt slices are
    # contiguous DMAs.
    wout = np.ascontiguousarray(
        W_out.T.reshape(KJ, 128, VT, 128).transpose(1, 2, 0, 3)).astype(f16)
    bias = np.ascontiguousarray(b_out.reshape(VT, 128).T).astype(np.float32)

    in_maps = []
    for b in range(B):
        encb = np.ascontiguousarray(
            enc_out[b].T.reshape(KE, 128, T).transpose(1, 0, 2)).astype(f16)
        predb = np.ascontiguousarray(
            pred_out[b].T.reshape(KE, 128, U).transpose(1, 0, 2)).astype(f16)
        in_maps.append({
            "enc": encb, "pred": predb, "wenc": wenc, "wpred": wpred,
            "wout": wout, "bias": bias,
        })
    return in_maps


def run(inputs, trace=False, **kwargs):
    from concourse.bass_utils import run_bass_kernel_spmd

    _enable_jax_compile_cache()
    nc = _get_compiled()
    in_maps = _prep_inputs(**inputs)
    res = run_bass_kernel_spmd(
        nc, in_maps, core_ids=list(range(N_CORES)), trace=trace, **kwargs)
    out = np.empty((B, T, U, VOCAB), np.float32)
    for b in range(B):
        q = res.results[b]["out"]                   # [VT, 128, U, T] fp16
        # out[b][t, u, vt*128+v] = q[vt, v, u, t]
        out[b] = q.transpose(3, 2, 0, 1).reshape(T, U, VOCAB)
    return out, res


def kernel(**inputs):
    out, _ = run(inputs, trace=False)
    return out


# revision 13
# speedup vs baseline: 1.1254x; 1.0126x over previous
"""RNN-T JointNetwork Trainium2 kernel.

logits[b,t,u,v] = sum_j W_out[v,j] * tanh(f[b,t,j] + g[b,u,j]) + b_out[v]
  f = enc_out @ W_enc.T   [B,T,640]
  g = pred_out @ W_pred.T [B,U,640]

Sharding: data-parallel over B=8 across the 8 NeuronCores (1 batch/core).

v3 design notes (vs the int8 baseline at 342 us):
  The baseline saturated all three compute engines (~90% each): the
  per-row int8 absmax/scale machinery cost VectorE+ScalarE as much time
  as the matmul itself, and tanh read fT from PSUM in f32 (ScalarE reads
  f32 at half rate). This version outputs fp16 (skips absmax entirely)
  and feeds tanh from fp16 SBUF, leaving the PE as the only near-
  saturated engine (bf16/fp16 matmul roofline ~277 us).

  phase 1: fT[j,t] (=f.T) accumulated in PSUM then drained to SBUF fp16;
           gT[j,u] drained f32 (used as tanh bias operand).
  phase 2: comb[j,uu,t] = tanh(fT + gT[:,u]) via ScalarE activation with
           per-partition bias, fp16 out; one tile per u-PAIR so a matmul
           can stream 512 moving columns (2 u x 256 t).
  phase 3: vocab-on-partition orientation: for each (ublock of 8 u, vt):
           out_ps[v,(uu,t)] += wout[j,vt-chunk].T @ comb[j] with j outer,
           pair inner -> each weight load feeds 4 N=512 matmuls (vs 1:2
           in the t-orientation), 4 psum banks x 2 generations = 8 banks.
  phase 4: VectorE drains each bank with ONE fused op: fp16 out =
           psum_f32 + bias_v (tensor_scalar add, per-partition bias),
           then DMA [128v, 2u x 256t] -> HBM [vt,v,u,t], 1 KiB contiguous
           per partition. Host transposes to [T,U,V] and upcasts.

  All matmul operands are fp16 (PE upconverts to e10m11 internally, so
  fp16 is strictly more accurate than bf16 at identical speed).
"""

import os
import sys

for _p in ("/opt/trn_rl_repo",):
    if _p not in sys.path:
        sys.path.insert(0, _p)

import numpy as np


def _enable_jax_compile_cache():
    """Persistent XLA executable cache: skips the per-process
    HLO->walrus->NEFF compile of the wrapped bass_exec call (~1s) when
    warm. jax is pre-imported by the site hook, so set config directly;
    cache errors are non-fatal to jax."""
    try:
        cc = os.path.expanduser("~/.cache/jax_bass_cc")
        os.makedirs(cc, exist_ok=True)
        import jax

        jax.config.update("jax_compilation_cache_dir", cc)
        jax.config.update("jax_persistent_cache_min_compile_time_secs", 0.0)
    except Exception:
        pass

B, T, U = 8, 256, 64
D_ENC, D_PRED, D_JOINT, VOCAB = 512, 512, 640, 1024
KE = D_ENC // 128   # 4 contraction chunks for enc/pred matmuls
KJ = D_JOINT // 128  # 5 contraction chunks for the vocab matmul
N_CORES = 8
VT = VOCAB // 128   # 8 vocab partition tiles
UB = 8              # u's per ublock
NP = UB // 2        # u-pairs per ublock

_compiled = None


def _build():
    import concourse.bacc as bacc
    import concourse.bass as bass
    import concourse.mybir as mybir
    import concourse.tile as tile

    f32 = mybir.dt.float32
    f16 = mybir.dt.float16
    PSUM = bass.MemorySpace.PSUM
    tanh = mybir.ActivationFunctionType.Tanh
    add = mybir.AluOpType.add

    nc = bacc.Bacc(
        "TRN2",
        target_bir_lowering=False,
        debug=False,
        enable_asserts=False,
    )

    enc_d = nc.dram_tensor("enc", [128, KE, T], f16, kind="ExternalInput")
    pred_d = nc.dram_tensor("pred", [128, KE, U], f16, kind="ExternalInput")
    wenc_d = nc.dram_tensor("wenc", [128, KJ, KE, 128], f16, kind="ExternalInput")
    wpred_d = nc.dram_tensor("wpred", [128, KJ, KE, 128], f16, kind="ExternalInput")
    wout_d = nc.dram_tensor("wout", [128, VT, KJ, 128], f16, kind="ExternalInput")
    bias_d = nc.dram_tensor("bias", [128, VT], f32, kind="ExternalInput")
    out_d = nc.dram_tensor("out", [VT, 128, U, T], f16, kind="ExternalOutput")

    out_qs = None  # round-robin DMA trigger engines for output tiles

    with tile.TileContext(nc) as tc:
        with (
            tc.tile_pool(name="const", bufs=1) as const,
            tc.tile_pool(name="comb", bufs=2) as comb_pool,
            tc.tile_pool(name="stage", bufs=3) as stage_pool,
        ):
            # Trigger the Tanh ACT table load before any data arrives.
            warm = const.tile([1, 8], f32)
            warm2 = const.tile([1, 8], f32)
            nc.vector.memset(warm[:], 0.0)
            nc.scalar.activation(warm2[:], warm[:], tanh)

            enc_sb = const.tile([128, KE, T], f16)
            pred_sb = const.tile([128, KE, U], f16)
            wenc_sb = const.tile([128, KJ, KE, 128], f16)
            wpred_sb = const.tile([128, KJ, KE, 128], f16)
            wout_sb = const.tile([128, VT, KJ, 128], f16)
            bias_sb = const.tile([128, VT], f32)
            fT_sb = const.tile([128, KJ, T], f16)
            gT_sb = const.tile([128, KJ, U], f32)

            # PE warmup: dummy matmuls on zeroed data while input DMAs are
            # in flight, so HAM un-throttles before the real matmuls start.
            wz = const.tile([128, 512], f16)
            nc.gpsimd.memset(wz[:], 0.0)

            # Input DMA triggers on sync+gpsimd only (scalar's FIFO must
            # stay clear for the tanh stream), chunked so each phase-1
            # j-group starts on its chunk's arrival; wout chunks trickle
            # in behind the critical wenc chunks in vt consumption order.
            nc.sync.dma_start(enc_sb[:], enc_d[:])
            nc.gpsimd.dma_start(pred_sb[:], pred_d[:])
            for j in range(KJ):
                nc.sync.dma_start(wenc_sb[:, j], wenc_d[:, j])
                nc.gpsimd.dma_start(wpred_sb[:, j], wpred_d[:, j])
            nc.gpsimd.dma_start(bias_sb[:], bias_d[:])
            for vt in range(VT):
                nc.sync.dma_start(wout_sb[:, vt], wout_d[:, vt])

            # 8 cold N=256 warmup matmuls bridge the ~3 us from preamble
            # end to the first input chunk arrival, keeping HAM busy.
            with tc.tile_pool(name="psw", bufs=1, space=PSUM) as psw:
                pw = psw.tile([128, 512], f32)
                for i in range(8):
                    nc.tensor.matmul(pw[:, 0:256], wz[:, :128], wz[:, 0:256],
                                     start=True, stop=True)

            # phase 1: per j-chunk, gT (small, drained while fT matmuls
            # run) then fT.
            with tc.tile_pool(name="psg", bufs=2, space=PSUM) as psg:
                for j in range(KJ):
                    psp = psg.tile([128, U], f32, tag="psp")
                    for k in range(KE):
                        nc.tensor.matmul(
                            psp[:],
                            wpred_sb[:, j, k, :],
                            pred_sb[:, k, :],
                            start=(k == 0),
                            stop=(k == KE - 1),
                        )
                    nc.vector.tensor_copy(gT_sb[:, j, :], psp[:])
                    psf = psg.tile([128, T], f32, tag="psf")
                    for k in range(KE):
                        nc.tensor.matmul(
                            psf[:],
                            wenc_sb[:, j, k, :],
                            enc_sb[:, k, :],
                            start=(k == 0),
                            stop=(k == KE - 1),
                        )
                    nc.vector.tensor_copy(fT_sb[:, j, :], psf[:])

            def make_comb(pairs):
                """Emit the tanh activations for the given u-pair indices;
                returns one tile [128, KJ, 2, T] fp16 per pair."""
                tiles = []
                for slot, q in enumerate(pairs):
                    cp = comb_pool.tile([128, KJ, 2, T], f16, tag=f"comb{slot}",
                                        name=f"comb{slot}_{q}")
                    for uu in range(2):
                        u = q * 2 + uu
                        for j in range(KJ):
                            nc.scalar.activation(
                                cp[:, j, uu, :],
                                fT_sb[:, j, :],
                                tanh,
                                bias=gT_sb[:, j, u:u + 1],
                            )
                    tiles.append(cp)
                return tiles

            # u-pair blocks: small leading blocks let phase 3 start after
            # only 10 activations instead of 40.
            blocks = [[0], [1], [2, 3]] + [
                [q, q + 1, q + 2, q + 3] for q in range(4, 32, 4)
            ]

            with tc.tile_pool(name="pso", bufs=2, space=PSUM) as pso:
                combs = make_comb(blocks[0])
                for bi, prs in enumerate(blocks):
                    last = bi == len(blocks) - 1
                    combs_next = None if last else make_comb(blocks[bi + 1])
                    for vt in range(VT):
                        pss = [
                            pso.tile([128, 512], f32, tag=f"ps{slot}",
                                     name=f"ps{slot}_{bi}_{vt}")
                            for slot in range(len(prs))
                        ]
                        for j in range(KJ):
                            w = wout_sb[:, vt, j, :]
                            for slot in range(len(prs)):
                                nc.tensor.matmul(
                                    pss[slot][:],
                                    w,
                                    combs[slot][:, j, :, :],
                                    start=(j == 0),
                                    stop=(j == KJ - 1),
                                )
                        for slot, q in enumerate(prs):
                            st = stage_pool.tile([128, 512], f16,
                                                 tag=f"st{slot}",
                                                 name=f"st{slot}_{bi}_{vt}")
                            nc.vector.tensor_scalar(
                                st[:], pss[slot][:], bias_sb[:, vt:vt + 1],
                                None, op0=add,
                            )
                            eng = (nc.sync, nc.gpsimd)[(vt + slot) % 2]
                            eng.dma_start(out_d[vt, :, 2 * q:2 * q + 2, :],
                                          st[:])
                    combs = combs_next

    nc.compile()
    return nc


def _get_compiled():
    global _compiled
    if _compiled is None:
        _compiled = _build()
    return _compiled


def _prep_inputs(enc_out, pred_out, W_enc, W_pred, W_out, b_out):
    f16 = np.float16
    enc_out = np.asarray(enc_out, dtype=np.float32)
    pred_out = np.asarray(pred_out, dtype=np.float32)
    W_enc = np.asarray(W_enc, dtype=np.float32)
    W_pred = np.asarray(W_pred, dtype=np.float32)
    W_out = np.asarray(W_out, dtype=np.float32)
    b_out = np.asarray(b_out, dtype=np.float32)

    # W_enc.T is [d, j]; chunk both axes by 128 -> [128 dp, KJ, KE, 128 jq]
    # (wenc[p, j, k, q] = W_enc[j*128+q, k*128+p]) so the per-j slice a
    # phase-1 group consumes is one contiguous DMA.
    wenc = np.ascontiguousarray(
        W_enc.T.reshape(KE, 128, KJ, 128).transpose(1, 2, 0, 3)).astype(f16)
    wpred = np.ascontiguousarray(
        W_pred.T.reshape(KE, 128, KJ, 128).transpose(1, 2, 0, 3)).astype(f16)
    # W_out.T is [j, v]; -> [128 jp, VT, KJ, 128 vq] so per-vt slices are
    # contiguous DMAs.
    wout = np.ascontiguousarray(
        W_out.T.reshape(KJ, 128, VT, 128).transpose(1, 2, 0, 3)).astype(f16)
    bias = np.ascontiguousarray(b_out.reshape(VT, 128).T).astype(np.float32)

    in_maps = []
    for b in range(B):
        encb = np.ascontiguousarray(
            enc_out[b].T.reshape(KE, 128, T).transpose(1, 0, 2)).astype(f16)
        predb = np.ascontiguousarray(
            pred_out[b].T.reshape(KE, 128, U).transpose(1, 0, 2)).astype(f16)
        in_maps.append({
            "enc": encb, "pred": predb, "wenc": wenc, "wpred": wpred,
            "wout": wout, "bias": bias,
        })
    return in_maps


def run(inputs, trace=False, **kwargs):
    from concourse.bass_utils import run_bass_kernel_spmd

    _enable_jax_compile_cache()
    nc = _get_compiled()
    in_maps = _prep_inputs(**inputs)
    res = run_bass_kernel_spmd(
        nc, in_maps, core_ids=list(range(N_CORES)), trace=trace, **kwargs)
    out = np.empty((B, T, U, VOCAB), np.float32)
    for b in range(B):
        q = res.results[b]["out"]                   # [VT, 128, U, T] fp16
        # out[b][t, u, vt*128+v] = q[vt, v, u, t]
        out[b] = q.transpose(3, 2, 0, 1).reshape(T, U, VOCAB)
    return out, res


def kernel(**inputs):
    out, _ = run(inputs, trace=False)
    return out


# revision 16
# speedup vs baseline: 1.1317x; 1.0056x over previous
"""RNN-T JointNetwork Trainium2 kernel.

logits[b,t,u,v] = sum_j W_out[v,j] * tanh(f[b,t,j] + g[b,u,j]) + b_out[v]
  f = enc_out @ W_enc.T   [B,T,640]
  g = pred_out @ W_pred.T [B,U,640]

Sharding: data-parallel over B=8 across the 8 NeuronCores (1 batch/core).

v3 design notes (vs the int8 baseline at 342 us):
  The baseline saturated all three compute engines (~90% each): the
  per-row int8 absmax/scale machinery cost VectorE+ScalarE as much time
  as the matmul itself, and tanh read fT from PSUM in f32 (ScalarE reads
  f32 at half rate). This version outputs fp16 (skips absmax entirely)
  and feeds tanh from fp16 SBUF, leaving the PE as the only near-
  saturated engine (bf16/fp16 matmul roofline ~277 us).

  phase 1: fT[j,t] (=f.T) accumulated in PSUM then drained to SBUF fp16;
           gT[j,u] drained f32 (used as tanh bias operand).
  phase 2: comb[j,uu,t] = tanh(fT + gT[:,u]) via ScalarE activation with
           per-partition bias, fp16 out; one tile per u-PAIR so a matmul
           can stream 512 moving columns (2 u x 256 t).
  phase 3: vocab-on-partition orientation: for each (ublock of 8 u, vt):
           out_ps[v,(uu,t)] += wout[j,vt-chunk].T @ comb[j] with j outer,
           pair inner -> each weight load feeds 4 N=512 matmuls (vs 1:2
           in the t-orientation), 4 psum banks x 2 generations = 8 banks.
  phase 4: VectorE drains each bank with ONE fused op: fp16 out =
           psum_f32 + bias_v (tensor_scalar add, per-partition bias),
           then DMA [128v, 2u x 256t] -> HBM [vt,v,u,t], 1 KiB contiguous
           per partition. Host transposes to [T,U,V] and upcasts.

  All matmul operands are fp16 (PE upconverts to e10m11 internally, so
  fp16 is strictly more accurate than bf16 at identical speed).
"""

import os
import sys

for _p in ("/opt/trn_rl_repo",):
    if _p not in sys.path:
        sys.path.insert(0, _p)

import numpy as np


def _enable_jax_compile_cache():
    """Persistent XLA executable cache: skips the per-process
    HLO->walrus->NEFF compile of the wrapped bass_exec call (~1s) when
    warm. jax is pre-imported by the site hook, so set config directly;
    cache errors are non-fatal to jax."""
    try:
        cc = os.path.expanduser("~/.cache/jax_bass_cc")
        os.makedirs(cc, exist_ok=True)
        import jax

        jax.config.update("jax_compilation_cache_dir", cc)
        jax.config.update("jax_persistent_cache_min_compile_time_secs", 0.0)
    except Exception:
        pass

B, T, U = 8, 256, 64
D_ENC, D_PRED, D_JOINT, VOCAB = 512, 512, 640, 1024
KE = D_ENC // 128   # 4 contraction chunks for enc/pred matmuls
KJ = D_JOINT // 128  # 5 contraction chunks for the vocab matmul
N_CORES = 8
VT = VOCAB // 128   # 8 vocab partition tiles
UB = 8              # u's per ublock
NP = UB // 2        # u-pairs per ublock

_compiled = None


def _build():
    import concourse.bacc as bacc
    import concourse.bass as bass
    import concourse.mybir as mybir
    import concourse.tile as tile

    f32 = mybir.dt.float32
    f16 = mybir.dt.float16
    PSUM = bass.MemorySpace.PSUM
    tanh = mybir.ActivationFunctionType.Tanh
    add = mybir.AluOpType.add

    nc = bacc.Bacc(
        "TRN2",
        target_bir_lowering=False,
        debug=False,
        enable_asserts=False,
    )

    enc_d = nc.dram_tensor("enc", [128, KE, T], f16, kind="ExternalInput")
    pred_d = nc.dram_tensor("pred", [128, KE, U], f16, kind="ExternalInput")
    wenc_d = nc.dram_tensor("wenc", [128, KJ, KE, 128], f16, kind="ExternalInput")
    wpred_d = nc.dram_tensor("wpred", [128, KJ, KE, 128], f16, kind="ExternalInput")
    wout_d = nc.dram_tensor("wout", [128, VT, KJ, 128], f16, kind="ExternalInput")
    bias_d = nc.dram_tensor("bias", [128, VT], f32, kind="ExternalInput")
    out_d = nc.dram_tensor("out", [VT, 128, U, T], f16, kind="ExternalOutput")

    out_qs = None  # round-robin DMA trigger engines for output tiles

    with tile.TileContext(nc) as tc:
        with (
            tc.tile_pool(name="const", bufs=1) as const,
            tc.tile_pool(name="comb", bufs=2) as comb_pool,
            tc.tile_pool(name="stage", bufs=3) as stage_pool,
        ):
            # Trigger the Tanh ACT table load before any data arrives.
            warm = const.tile([1, 8], f32)
            warm2 = const.tile([1, 8], f32)
            nc.vector.memset(warm[:], 0.0)
            nc.scalar.activation(warm2[:], warm[:], tanh)

            enc_sb = const.tile([128, KE, T], f16)
            pred_sb = const.tile([128, KE, U], f16)
            wenc_sb = const.tile([128, KJ, KE, 128], f16)
            wpred_sb = const.tile([128, KJ, KE, 128], f16)
            wout_sb = const.tile([128, VT, KJ, 128], f16)
            bias_sb = const.tile([128, VT], f32)
            fT_sb = const.tile([128, KJ, T], f16)
            gT_sb = const.tile([128, KJ, U], f32)

            # PE warmup: dummy matmuls on zeroed data while input DMAs are
            # in flight, so HAM un-throttles before the real matmuls start.
            wz = const.tile([128, 512], f16)
            nc.gpsimd.memset(wz[:], 0.0)

            # Input DMA triggers on sync+gpsimd only (scalar's FIFO must
            # stay clear for the tanh stream), chunked so each phase-1
            # j-group starts on its chunk's arrival; wout chunks trickle
            # in behind the critical wenc chunks in vt consumption order.
            nc.sync.dma_start(enc_sb[:, 0:1], enc_d[:, 0:1])
            nc.gpsimd.dma_start(pred_sb[:], pred_d[:])
            nc.sync.dma_start(wenc_sb[:, 0], wenc_d[:, 0])
            nc.sync.dma_start(enc_sb[:, 1:], enc_d[:, 1:])
            for j in range(KJ):
                nc.gpsimd.dma_start(wpred_sb[:, j], wpred_d[:, j])
                if j > 0:
                    nc.sync.dma_start(wenc_sb[:, j], wenc_d[:, j])
            nc.gpsimd.dma_start(bias_sb[:], bias_d[:])
            for vt in range(VT):
                nc.gpsimd.dma_start(wout_sb[:, vt], wout_d[:, vt])

            # Cold N=256 warmup matmuls bridge the ~2 us from preamble
            # end to the first input chunk arrival, keeping HAM busy.
            with tc.tile_pool(name="psw", bufs=1, space=PSUM) as psw:
                pw = psw.tile([128, 512], f32)
                for i in range(6):
                    nc.tensor.matmul(pw[:, 0:256], wz[:, :128], wz[:, 0:256],
                                     start=True, stop=True)

            # phase 1: per j-chunk, gT (small, drained while fT matmuls
            # run) then fT.
            with tc.tile_pool(name="psg", bufs=2, space=PSUM) as psg:
                for j in range(KJ):
                    psp = psg.tile([128, U], f32, tag="psp")
                    for k in range(KE):
                        nc.tensor.matmul(
                            psp[:],
                            wpred_sb[:, j, k, :],
                            pred_sb[:, k, :],
                            start=(k == 0),
                            stop=(k == KE - 1),
                        )
                    nc.vector.tensor_copy(gT_sb[:, j, :], psp[:])
                    psf = psg.tile([128, T], f32, tag="psf")
                    for k in range(KE):
                        nc.tensor.matmul(
                            psf[:],
                            wenc_sb[:, j, k, :],
                            enc_sb[:, k, :],
                            start=(k == 0),
                            stop=(k == KE - 1),
                        )
                    nc.vector.tensor_copy(fT_sb[:, j, :], psf[:])

            def make_comb(pairs):
                """Emit the tanh activations for the given u-pair indices;
                returns one tile [128, KJ, 2, T] fp16 per pair."""
                tiles = []
                for slot, q in enumerate(pairs):
                    cp = comb_pool.tile([128, KJ, 2, T], f16, tag=f"comb{slot}",
                                        name=f"comb{slot}_{q}")
                    for uu in range(2):
                        u = q * 2 + uu
                        for j in range(KJ):
                            nc.scalar.activation(
                                cp[:, j, uu, :],
                                fT_sb[:, j, :],
                                tanh,
                                bias=gT_sb[:, j, u:u + 1],
                            )
                    tiles.append(cp)
                return tiles

            # u-pair blocks: small leading blocks let phase 3 start after
            # only 10 activations instead of 40; small trailing blocks
            # shrink the serial drain tail after the last matmul.
            blocks = [[0], [1], [2, 3]] + [
                [q, q + 1, q + 2, q + 3] for q in range(4, 28, 4)
            ] + [[28, 29], [30], [31]]

            with tc.tile_pool(name="pso", bufs=2, space=PSUM) as pso:
                combs = make_comb(blocks[0])
                for bi, prs in enumerate(blocks):
                    last = bi == len(blocks) - 1
                    combs_next = None if last else make_comb(blocks[bi + 1])
                    for vt in range(VT):
                        pss = [
                            pso.tile([128, 512], f32, tag=f"ps{slot}",
                                     name=f"ps{slot}_{bi}_{vt}")
                            for slot in range(len(prs))
                        ]
                        for j in range(KJ):
                            w = wout_sb[:, vt, j, :]
                            for slot in range(len(prs)):
                                nc.tensor.matmul(
                                    pss[slot][:],
                                    w,
                                    combs[slot][:, j, :, :],
                                    start=(j == 0),
                                    stop=(j == KJ - 1),
                                )
                        for slot, q in enumerate(prs):
                            st = stage_pool.tile([128, 512], f16,
                                                 tag=f"st{slot}",
                                                 name=f"st{slot}_{bi}_{vt}")
                            nc.vector.tensor_scalar(
                                st[:], pss[slot][:], bias_sb[:, vt:vt + 1],
                                None, op0=add,
                            )
                            nc.sync.dma_start(out_d[vt, :, 2 * q:2 * q + 2, :],
                                              st[:])
                    combs = combs_next

    nc.compile()
    return nc


def _get_compiled():
    global _compiled
    if _compiled is None:
        _compiled = _build()
    return _compiled


def _prep_inputs(enc_out, pred_out, W_enc, W_pred, W_out, b_out):
    f16 = np.float16
    enc_out = np.asarray(enc_out, dtype=np.float32)
    pred_out = np.asarray(pred_out, dtype=np.float32)
    W_enc = np.asarray(W_enc, dtype=np.float32)
    W_pred = np.asarray(W_pred, dtype=np.float32)
    W_out = np.asarray(W_out, dtype=np.float32)
    b_out = np.asarray(b_out, dtype=np.float32)

    # W_enc.T is [d, j]; chunk both axes by 128 -> [128 dp, KJ, KE, 128 jq]
    # (wenc[p, j, k, q] = W_enc[j*128+q, k*128+p]) so the per-j slice a
    # phase-1 group consumes is one contiguous DMA.
    wenc = np.ascontiguousarray(
        W_enc.T.reshape(KE, 128, KJ, 128).transpose(1, 2, 0, 3)).astype(f16)
    wpred = np.ascontiguousarray(
        W_pred.T.reshape(KE, 128, KJ, 128).transpose(1, 2, 0, 3)).astype(f16)
    # W_out.T is [j, v]; -> [128 jp, VT, KJ, 128 vq] so per-vt slices are
    # contiguous DMAs.
    wout = np.ascontiguousarray(
        W_out.T.reshape(KJ, 128, VT, 128).transpose(1, 2, 0, 3)).astype(f16)
    bias = np.ascontiguousarray(b_out.reshape(VT, 128).T).astype(np.float32)

    in_maps = []
    for b in range(B):
        encb = np.ascontiguousarray(
            enc_out[b].T.reshape(KE, 128, T).transpose(1, 0, 2)).astype(f16)
        predb = np.ascontiguousarray(
            pred_out[b].T.reshape(KE, 128, U).transpose(1, 0, 2)).astype(f16)
        in_maps.append({
            "enc": encb, "pred": predb, "wenc": wenc, "wpred": wpred,
            "wout": wout, "bias": bias,
        })
    return in_maps


def run(inputs, trace=False, **kwargs):
    from concourse.bass_utils import run_bass_kernel_spmd

    _enable_jax_compile_cache()
    nc = _get_compiled()
    in_maps = _prep_inputs(**inputs)
    res = run_bass_kernel_spmd(
        nc, in_maps, core_ids=list(range(N_CORES)), trace=trace, **kwargs)
    out = np.empty((B, T, U, VOCAB), np.float32)
    for b in range(B):
        q = res.results[b]["out"]                   # [VT, 128, U, T] fp16
        # out[b][t, u, vt*128+v] = q[vt, v, u, t]
        out[b] = q.transpose(3, 2, 0, 1).reshape(T, U, VOCAB)
    return out, res


def kernel(**inputs):
    out, _ = run(inputs, trace=False)
    return out


# revision 18
# speedup vs baseline: 1.1339x; 1.0020x over previous
"""RNN-T JointNetwork Trainium2 kernel.

logits[b,t,u,v] = sum_j W_out[v,j] * tanh(f[b,t,j] + g[b,u,j]) + b_out[v]
  f = enc_out @ W_enc.T   [B,T,640]
  g = pred_out @ W_pred.T [B,U,640]

Sharding: data-parallel over B=8 across the 8 NeuronCores (1 batch/core).

v3 design notes (vs the int8 baseline at 342 us):
  The baseline saturated all three compute engines (~90% each): the
  per-row int8 absmax/scale machinery cost VectorE+ScalarE as much time
  as the matmul itself, and tanh read fT from PSUM in f32 (ScalarE reads
  f32 at half rate). This version outputs fp16 (skips absmax entirely)
  and feeds tanh from fp16 SBUF, leaving the PE as the only near-
  saturated engine (bf16/fp16 matmul roofline ~277 us).

  phase 1: fT[j,t] (=f.T) accumulated in PSUM then drained to SBUF fp16;
           gT[j,u] drained f32 (used as tanh bias operand).
  phase 2: comb[j,uu,t] = tanh(fT + gT[:,u]) via ScalarE activation with
           per-partition bias, fp16 out; one tile per u-PAIR so a matmul
           can stream 512 moving columns (2 u x 256 t).
  phase 3: vocab-on-partition orientation: for each (ublock of 8 u, vt):
           out_ps[v,(uu,t)] += wout[j,vt-chunk].T @ comb[j] with j outer,
           pair inner -> each weight load feeds 4 N=512 matmuls (vs 1:2
           in the t-orientation), 4 psum banks x 2 generations = 8 banks.
  phase 4: VectorE drains each bank with ONE fused op: fp16 out =
           psum_f32 + bias_v (tensor_scalar add, per-partition bias),
           then DMA [128v, 2u x 256t] -> HBM [vt,v,u,t], 1 KiB contiguous
           per partition. Host transposes to [T,U,V] and upcasts.

  All matmul operands are fp16 (PE upconverts to e10m11 internally, so
  fp16 is strictly more accurate than bf16 at identical speed).
"""

import os
import sys

for _p in ("/opt/trn_rl_repo",):
    if _p not in sys.path:
        sys.path.insert(0, _p)

import numpy as np


def _enable_jax_compile_cache():
    """Persistent XLA executable cache: skips the per-process
    HLO->walrus->NEFF compile of the wrapped bass_exec call (~1s) when
    warm. jax is pre-imported by the site hook, so set config directly;
    cache errors are non-fatal to jax."""
    try:
        cc = os.path.expanduser("~/.cache/jax_bass_cc")
        os.makedirs(cc, exist_ok=True)
        import jax

        jax.config.update("jax_compilation_cache_dir", cc)
        jax.config.update("jax_persistent_cache_min_compile_time_secs", 0.0)
    except Exception:
        pass

B, T, U = 8, 256, 64
D_ENC, D_PRED, D_JOINT, VOCAB = 512, 512, 640, 1024
KE = D_ENC // 128   # 4 contraction chunks for enc/pred matmuls
KJ = D_JOINT // 128  # 5 contraction chunks for the vocab matmul
N_CORES = 8
VT = VOCAB // 128   # 8 vocab partition tiles
UB = 8              # u's per ublock
NP = UB // 2        # u-pairs per ublock

_compiled = None


def _build():
    import concourse.bacc as bacc
    import concourse.bass as bass
    import concourse.mybir as mybir
    import concourse.tile as tile

    f32 = mybir.dt.float32
    f16 = mybir.dt.float16
    PSUM = bass.MemorySpace.PSUM
    tanh = mybir.ActivationFunctionType.Tanh
    add = mybir.AluOpType.add

    nc = bacc.Bacc(
        "TRN2",
        target_bir_lowering=False,
        debug=False,
        enable_asserts=False,
    )

    enc_d = nc.dram_tensor("enc", [128, KE, T], f16, kind="ExternalInput")
    pred_d = nc.dram_tensor("pred", [128, KE, U], f16, kind="ExternalInput")
    wenc_d = nc.dram_tensor("wenc", [128, KJ, KE, 128], f16, kind="ExternalInput")
    wpred_d = nc.dram_tensor("wpred", [128, KJ, KE, 128], f16, kind="ExternalInput")
    wout_d = nc.dram_tensor("wout", [128, VT, KJ, 128], f16, kind="ExternalInput")
    bias_d = nc.dram_tensor("bias", [128, VT], f32, kind="ExternalInput")
    out_d = nc.dram_tensor("out", [VT, 128, U, T], f16, kind="ExternalOutput")

    out_qs = None  # round-robin DMA trigger engines for output tiles

    with tile.TileContext(nc) as tc:
        with (
            tc.tile_pool(name="const", bufs=1) as const,
            tc.tile_pool(name="comb", bufs=2) as comb_pool,
            tc.tile_pool(name="stage", bufs=3) as stage_pool,
        ):
            # Trigger the Tanh ACT table load before any data arrives.
            warm = const.tile([1, 8], f32)
            warm2 = const.tile([1, 8], f32)
            nc.vector.memset(warm[:], 0.0)
            nc.scalar.activation(warm2[:], warm[:], tanh)

            enc_sb = const.tile([128, KE, T], f16)
            pred_sb = const.tile([128, KE, U], f16)
            wenc_sb = const.tile([128, KJ, KE, 128], f16)
            wpred_sb = const.tile([128, KJ, KE, 128], f16)
            wout_sb = const.tile([128, VT, KJ, 128], f16)
            bias_sb = const.tile([128, VT], f32)
            fT_sb = const.tile([128, KJ, T], f16)
            gT_sb = const.tile([128, KJ, U], f32)

            # PE warmup: dummy matmuls on zeroed data while input DMAs are
            # in flight, so HAM un-throttles before the real matmuls start.
            wz = const.tile([128, 512], f16)
            nc.gpsimd.memset(wz[:], 0.0)

            # Input DMA triggers on sync+gpsimd only (scalar's FIFO must
            # stay clear for the tanh stream), chunked so each phase-1
            # j-group starts on its chunk's arrival; wout chunks trickle
            # in behind the critical wenc chunks in vt consumption order.
            nc.sync.dma_start(enc_sb[:, 0:1], enc_d[:, 0:1])
            nc.gpsimd.dma_start(pred_sb[:], pred_d[:])
            nc.sync.dma_start(wenc_sb[:, 0], wenc_d[:, 0])
            nc.sync.dma_start(enc_sb[:, 1:], enc_d[:, 1:])
            for j in range(KJ):
                nc.gpsimd.dma_start(wpred_sb[:, j], wpred_d[:, j])
                if j > 0:
                    nc.sync.dma_start(wenc_sb[:, j], wenc_d[:, j])
            nc.gpsimd.dma_start(bias_sb[:], bias_d[:])
            for vt in range(VT):
                nc.gpsimd.dma_start(wout_sb[:, vt], wout_d[:, vt])

            # Cold N=256 warmup matmuls bridge the ~2 us from preamble
            # end to the first input chunk arrival, keeping HAM busy.
            with tc.tile_pool(name="psw", bufs=1, space=PSUM) as psw:
                pw = psw.tile([128, 512], f32)
                for i in range(5):
                    nc.tensor.matmul(pw[:, 0:256], wz[:, :128], wz[:, 0:256],
                                     start=True, stop=True)

            # phase 1: per j-chunk, gT (small, drained while fT matmuls
            # run) then fT.
            with tc.tile_pool(name="psg", bufs=2, space=PSUM) as psg:
                for j in range(KJ):
                    psp = psg.tile([128, U], f32, tag="psp")
                    for k in range(KE):
                        nc.tensor.matmul(
                            psp[:],
                            wpred_sb[:, j, k, :],
                            pred_sb[:, k, :],
                            start=(k == 0),
                            stop=(k == KE - 1),
                        )
                    nc.vector.tensor_copy(gT_sb[:, j, :], psp[:])
                    psf = psg.tile([128, T], f32, tag="psf")
                    for k in range(KE):
                        nc.tensor.matmul(
                            psf[:],
                            wenc_sb[:, j, k, :],
                            enc_sb[:, k, :],
                            start=(k == 0),
                            stop=(k == KE - 1),
                        )
                    nc.vector.tensor_copy(fT_sb[:, j, :], psf[:])

            def make_comb(pairs):
                """Emit the tanh activations for the given u-pair indices;
                returns one tile [128, KJ, 2, T] fp16 per pair."""
                tiles = []
                for slot, q in enumerate(pairs):
                    cp = comb_pool.tile([128, KJ, 2, T], f16, tag=f"comb{slot}",
                                        name=f"comb{slot}_{q}")
                    for uu in range(2):
                        u = q * 2 + uu
                        for j in range(KJ):
                            nc.scalar.activation(
                                cp[:, j, uu, :],
                                fT_sb[:, j, :],
                                tanh,
                                bias=gT_sb[:, j, u:u + 1],
                            )
                    tiles.append(cp)
                return tiles

            # u-pair blocks: the leading ramp is sized so each block's
            # tanh activations fit inside the previous block's matmul
            # span; small trailing blocks shrink the serial drain tail
            # after the last matmul.
            blocks = [[0], [1], [2, 3], [4, 5, 6]] + [
                [q, q + 1, q + 2, q + 3] for q in range(7, 27, 4)
            ] + [[27, 28], [29, 30], [31]]

            with tc.tile_pool(name="pso", bufs=2, space=PSUM) as pso:
                combs = make_comb(blocks[0])
                for bi, prs in enumerate(blocks):
                    last = bi == len(blocks) - 1
                    combs_next = None if last else make_comb(blocks[bi + 1])
                    for vt in range(VT):
                        pss = [
                            pso.tile([128, 512], f32, tag=f"ps{slot}",
                                     name=f"ps{slot}_{bi}_{vt}")
                            for slot in range(len(prs))
                        ]
                        for j in range(KJ):
                            w = wout_sb[:, vt, j, :]
                            for slot in range(len(prs)):
                                nc.tensor.matmul(
                                    pss[slot][:],
                                    w,
                                    combs[slot][:, j, :, :],
                                    start=(j == 0),
                                    stop=(j == KJ - 1),
                                )
                        for slot, q in enumerate(prs):
                            st = stage_pool.tile([128, 512], f16,
                                                 tag=f"st{slot}",
                                                 name=f"st{slot}_{bi}_{vt}")
                            nc.vector.tensor_scalar(
                                st[:], pss[slot][:], bias_sb[:, vt:vt + 1],
                                None, op0=add,
                            )
                            nc.sync.dma_start(out_d[vt, :, 2 * q:2 * q + 2, :],
                                              st[:])
                    combs = combs_next

    nc.compile()
    return nc


def _get_compiled():
    global _compiled
    if _compiled is None:
        _compiled = _build()
    return _compiled


def _prep_inputs(enc_out, pred_out, W_enc, W_pred, W_out, b_out):
    f16 = np.float16
    enc_out = np.asarray(enc_out, dtype=np.float32)
    pred_out = np.asarray(pred_out, dtype=np.float32)
    W_enc = np.asarray(W_enc, dtype=np.float32)
    W_pred = np.asarray(W_pred, dtype=np.float32)
    W_out = np.asarray(W_out, dtype=np.float32)
    b_out = np.asarray(b_out, dtype=np.float32)

    # W_enc.T is [d, j]; chunk both axes by 128 -> [128 dp, KJ, KE, 128 jq]
    # (wenc[p, j, k, q] = W_enc[j*128+q, k*128+p]) so the per-j slice a
    # phase-1 group consumes is one contiguous DMA.
    wenc = np.ascontiguousarray(
        W_enc.T.reshape(KE, 128, KJ, 128).transpose(1, 2, 0, 3)).astype(f16)
    wpred = np.ascontiguousarray(
        W_pred.T.reshape(KE, 128, KJ, 128).transpose(1, 2, 0, 3)).astype(f16)
    # W_out.T is [j, v]; -> [128 jp, VT, KJ, 128 vq] so per-vt slices are
    # contiguous DMAs.
    wout = np.ascontiguousarray(
        W_out.T.reshape(KJ, 128, VT, 128).transpose(1, 2, 0, 3)).astype(f16)
    bias = np.ascontiguousarray(b_out.reshape(VT, 128).T).astype(np.float32)

    in_maps = []
    for b in range(B):
        encb = np.ascontiguousarray(
            enc_out[b].T.reshape(KE, 128, T).transpose(1, 0, 2)).astype(f16)
        predb = np.ascontiguousarray(
            pred_out[b].T.reshape(KE, 128, U).transpose(1, 0, 2)).astype(f16)
        in_maps.append({
            "enc": encb, "pred": predb, "wenc": wenc, "wpred": wpred,
            "wout": wout, "bias": bias,
        })
    return in_maps


def run(inputs, trace=False, **kwargs):
    from concourse.bass_utils import run_bass_kernel_spmd

    _enable_jax_compile_cache()
    nc = _get_compiled()
    in_maps = _prep_inputs(**inputs)
    res = run_bass_kernel_spmd(
        nc, in_maps, core_ids=list(range(N_CORES)), trace=trace, **kwargs)
    out = np.empty((B, T, U, VOCAB), np.float32)
    for b in range(B):
        q = res.results[b]["out"]                   # [VT, 128, U, T] fp16
        # out[b][t, u, vt*128+v] = q[vt, v, u, t]
        out[b] = q.transpose(3, 2, 0, 1).reshape(T, U, VOCAB)
    return out, res


def kernel(**inputs):
    out, _ = run(inputs, trace=False)
    return out
